# revision 66
# baseline (speedup 1.0000x reference)
"""Causal multi-head attention (B=4, T=2048, D=1024, H=16) on 8 trn2 cores.

Sharding: core c -> (batch b = c//2, head-group g = c%2) -> 8 heads/core.
Per-core Bass kernel: QKV projections, causal flash attention with
transposed scores (s^T = K @ Q^T) but q-major AV accumulation
(out[q, d] = ex^T V via ex-as-stationary matmuls, N=65 with an appended
ones column in V giving the softmax denominator per psum partition).
Normalization folds into PSUM eviction (reciprocal_approx_fast +
per-partition tensor_scalar_mul), then a PE transpose restores d-major
layout for the output projection. The attention inner loop is ACT
(exp) throughput bound, so V projection and Q/K projections are diced
into small matmul "pieces" pumped into the PE's slack between chunks.
Host sums the two head-group partials per batch (row-parallel proj).
"""

import numpy as np
import ml_dtypes

import concourse.bass as bass  # noqa: F401  (bass types via bacc)
import concourse.bacc as bacc
import concourse.mybir as mybir
import concourse.tile as tile
from concourse.bass_utils import run_bass_kernel_spmd

B, T, D = 4, 2048, 1024
H, DH = 16, 64
N_CORES = 8
HPC = 8      # heads per core
PAIRS = HPC // 2
BF = mybir.dt.bfloat16
F32 = mybir.dt.float32
BF_NP = ml_dtypes.bfloat16

TQ = 512     # q block (free dim)
TK = 128     # k block (partition dim)
NQG = T // TQ
NKC = T // TK
QB = TQ // TK   # 128-wide q sub-blocks per q group


def build_nc():
    nc = bacc.Bacc(
        "TRN2",
        target_bir_lowering=False,
        debug=False,
        enable_asserts=True,
        num_devices=N_CORES,
    )
    xT = nc.dram_tensor("xT", [D, T], BF, kind="ExternalInput")
    wq = nc.dram_tensor("wq", [D, 512], BF, kind="ExternalInput")
    wk = nc.dram_tensor("wk", [D, 512], BF, kind="ExternalInput")
    wv = nc.dram_tensor("wv", [D, 512], BF, kind="ExternalInput")
    wp = nc.dram_tensor("wp", [512, D], BF, kind="ExternalInput")
    ident = nc.dram_tensor("ident", [128, 128], BF, kind="ExternalInput")
    y = nc.dram_tensor("y", [T, D], BF, kind="ExternalOutput")

    with tile.TileContext(nc) as tc:
        with (
            tc.tile_pool(name="pers", bufs=1) as pers,
            tc.tile_pool(name="work", bufs=1) as work,
            tc.tile_pool(name="ps", bufs=1, space="PSUM") as pp,
        ):
            # ---- persistent SBUF (per-dc tiles => DMA-granular deps) ----
            # xT split by (d-chunk, token-quarter) so q-group 0 compute can
            # start as soon as the first quarter lands
            xT_t = [[pers.tile([128, 512], BF, tag=f"xT{dc}_{tq}",
                               name=f"xT{dc}_{tq}") for tq in range(4)]
                    for dc in range(8)]
            wq_t = [pers.tile([128, 512], BF, tag=f"wq{dc}", name=f"wq{dc}")
                    for dc in range(8)]
            wk_t = [pers.tile([128, 512], BF, tag=f"wk{dc}", name=f"wk{dc}")
                    for dc in range(8)]
            wv_t = [pers.tile([128, 512], BF, tag=f"wv{dc}", name=f"wv{dc}")
                    for dc in range(8)]
            wp_sb = pers.tile([128, 4, D], BF, tag="wp", name="wp_sb")
            id_sb = pers.tile([128, 128], BF, tag="id", name="id_sb")
            # V in token-major layout with a ones column per head: [tok, head, 65]
            vext = pers.tile([128, NKC, HPC, 65], BF, tag="vext", name="vext")
            # normalized attention outputs, d-major: [pair-chan, pair, tok]
            outT = pers.tile([128, PAIRS, T], BF, tag="outT", name="outT")
            # causal mask for diagonal blocks: keep q >= k
            mask_sb = pers.tile([128, 128], BF, tag="mask", name="mask_sb")

            # ---- loads, chunk-interleaved so compute starts early ----
            # spread DMA triggers across engine queues (trigger issue is the
            # serial bottleneck, ~0.6us per dma_start on one queue)
            nc.sync.dma_start(id_sb[:, :], ident[:, :])
            for dc in range(8):
                nc.scalar.dma_start(wq_t[dc][:, :],
                                    wq[dc * 128:(dc + 1) * 128, :])
                nc.sync.dma_start(wk_t[dc][:, :],
                                  wk[dc * 128:(dc + 1) * 128, :])
                nc.gpsimd.dma_start(
                    xT_t[dc][0][:, :], xT[dc * 128:(dc + 1) * 128, 0:512])
            for dc in range(8):
                nc.scalar.dma_start(wv_t[dc][:, :],
                                    wv[dc * 128:(dc + 1) * 128, :])
                nc.gpsimd.dma_start(
                    xT_t[dc][1][:, :], xT[dc * 128:(dc + 1) * 128, 512:1024])
            for tq in range(2, 4):
                for dc in range(8):
                    eng = nc.sync if tq == 2 else nc.gpsimd
                    eng.dma_start(
                        xT_t[dc][tq][:, :],
                        xT[dc * 128:(dc + 1) * 128, tq * 512:(tq + 1) * 512])
            for cc in range(4):
                nc.sync.dma_start(wp_sb[:, cc, :],
                                  wp[cc * 128:(cc + 1) * 128, :])
            # HAM warmup: keep the PE clock-gate open across the DMA wait
            # with dependency-free dummy matmuls (sources built by memset)
            warm_w = work.tile([128, 512], BF, tag="warm", bufs=1,
                               name="warm_w")
            nc.vector.memset(warm_w[:, :], 0.5)
            ps_w = pp.tile([128, 512], F32, tag="pq", bufs=1, name="ps_w")
            for i in range(44):
                nc.tensor.matmul(
                    ps_w[:, :], warm_w[:, 0:128], warm_w[:, :],
                    start=True, stop=True, skip_group_check=True,
                )
            nc.gpsimd.memset(vext[:, :, :, 64], 1.0)
            nc.gpsimd.memset(mask_sb[:, :], 1.0)
            nc.gpsimd.affine_select(
                mask_sb[:, :],
                mask_sb[:, :],
                pattern=[[1, 128]],
                compare_op=mybir.AluOpType.is_ge,
                fill=0.0,
                base=0,
                channel_multiplier=-1,
            )

            # ---- background work pieces (V proj, Q/K proj) ----
            pair_qt = {}
            pair_kt = {}
            v_done = set()
            proj_done = set()

            def emit_v(tk):
                """V chunk tk: vext[:, tk] = (x @ wv)[tk block], + ones col."""
                if tk in v_done:
                    return
                v_done.add(tk)
                ps_v = pp.tile([128, 512], F32, tag="pq", bufs=1, name="ps_v")
                tq, to = tk // 4, (tk % 4) * 128
                for dc in range(8):
                    nc.tensor.matmul(
                        ps_v[:, :],
                        xT_t[dc][tq][:, to:to + 128],
                        wv_t[dc][:, :],
                        start=(dc == 0),
                        stop=(dc == 7),
                        skip_group_check=True,
                    )
                nc.vector.tensor_copy(
                    vext[:, tk, :, 0:64],
                    ps_v.rearrange("p (h d) -> p h d", d=64),
                )

            def emit_proj(hp, qg, which):
                """Q^T or K^T for (pair hp, q-group qg), d-major.

                Rows = pair channels: head0 d 0-63 on partitions 0-63,
                head1 d 0-63 on partitions 64-127.
                """
                if (hp, qg, which) in proj_done:
                    return
                proj_done.add((hp, qg, which))
                if hp not in pair_qt:
                    pair_qt[hp] = work.tile([128, T], BF, tag="qt", bufs=2,
                                            name="qt")
                    pair_kt[hp] = work.tile([128, T], BF, tag="kt", bufs=2,
                                            name="kt")
                dst = pair_qt[hp] if which == "q" else pair_kt[hp]
                w_t = wq_t if which == "q" else wk_t
                ps_p = pp.tile([128, 512], F32, tag="pq", bufs=1, name="ps_p")
                for dc in range(8):
                    nc.tensor.matmul(
                        ps_p[:, :],
                        w_t[dc][:, hp * 128:(hp + 1) * 128],
                        xT_t[dc][qg][:, :],
                        start=(dc == 0),
                        stop=(dc == 7),
                        skip_group_check=True,
                    )
                nc.vector.tensor_copy(dst[:, qg * TQ:(qg + 1) * TQ], ps_p[:, :])

            queue = []

            def pump(n):
                for _ in range(min(n, len(queue))):
                    queue.pop(0)()

            # ---- phase-3 pieces: y[tk block] = outT.T @ wp, pumped into
            # pair 3's slack once the needed outT blocks are final ----
            y_done = set()

            def emit_y(tk, nb):
                if (tk, nb) in y_done:
                    return
                y_done.add((tk, nb))
                ps_y = pp.tile([128, 512], F32, tag="pq", bufs=1, name="ps_y")
                for cc in range(4):
                    nc.tensor.matmul(
                        ps_y[:, :],
                        outT[:, cc, tk * 128:(tk + 1) * 128],
                        wp_sb[:, cc, nb * 512:(nb + 1) * 512],
                        start=(cc == 0),
                        stop=(cc == 3),
                        skip_group_check=True,
                    )
                y_ev = work.tile([128, 512], BF, tag="yev", bufs=3,
                                 name="y_ev")
                sl = slice(nb * 512, (nb + 1) * 512)
                if (tk + nb) % 2 == 0:
                    nc.vector.tensor_copy(y_ev[:, :], ps_y[:, :])
                    nc.sync.dma_start(
                        y[tk * 128:(tk + 1) * 128, sl], y_ev[:, :])
                else:
                    nc.scalar.copy(y_ev[:, :], ps_y[:, :])
                    nc.gpsimd.dma_start(
                        y[tk * 128:(tk + 1) * 128, sl], y_ev[:, :])

            # ---- phase 2 ----
            emit_proj(0, 0, "q")
            emit_proj(0, 0, "k")

            for hp in range(PAIRS):
                for qg in range(NQG):
                    kmax = (qg + 1) * QB
                    noff = qg * QB
                    # overdue pieces first (deps of this block)
                    pump(len(queue))
                    emit_proj(hp, qg, "q")
                    emit_proj(hp, qg, "k")
                    # enqueue pieces for the next block
                    if qg + 1 < NQG:
                        nhp, nqg = hp, qg + 1
                    elif hp + 1 < PAIRS:
                        nhp, nqg = hp + 1, 0
                    else:
                        nhp = None
                    if nhp is not None:
                        queue.append(lambda a=nhp, b=nqg: emit_proj(a, b, "q"))
                        queue.append(lambda a=nhp, b=nqg: emit_proj(a, b, "k"))
                        if nhp == 0:
                            for tk in range(kmax, (nqg + 1) * QB):
                                queue.append(lambda t=tk: emit_v(t))
                    if hp == PAIRS - 1 and qg >= 1:
                        # y blocks of the previous q-group are final now
                        for tk in range((qg - 1) * QB, qg * QB):
                            for nb in range(2):
                                queue.append(lambda t=tk, n=nb: emit_y(t, n))

                    qt, kt = pair_qt[hp], pair_kt[hp]
                    # unnormalized AV accumulators, q-major:
                    # region (h, qb) = psO[h][:, qb, 0:64] + den col 64
                    psO = [
                        pp.tile([128, QB, 65], F32, tag=f"av{h}", bufs=1,
                                name=f"psO{h}")
                        for h in range(2)
                    ]
                    # transposed normalized outputs [d, (h qb), q]
                    psT = pp.tile([64, 2 * QB, 128], BF, tag="tp", bufs=1,
                                  name="psT")

                    def qk(kc):
                        # scores^T chunk for both heads: [k 128, q 512] x2
                        # on diagonal blocks only columns q >= j*128 live
                        off = max(0, kc - noff) * TK
                        ps_s = pp.tile([128, 1024], F32, tag="sc", bufs=2,
                                       name="ps_s")
                        for h in (0, 1):
                            nc.tensor.matmul(
                                ps_s[:, h * 512 + off:(h + 1) * 512],
                                kt[h * 64:(h + 1) * 64, kc * TK:(kc + 1) * TK],
                                qt[h * 64:(h + 1) * 64,
                                   qg * TQ + off:(qg + 1) * TQ],
                                start=True, stop=True,
                                skip_group_check=True,
                            )
                        return ps_s

                    def evict(h, qb, pO, pT):
                        # normalize region (h, qb) and transpose to psT
                        den_r = work.tile([128, 1], F32, tag="denr", bufs=4,
                                          name="den_r")
                        nc.vector.reciprocal_approx_fast(
                            den_r[:, :], pO[h][:, qb, 64:65])
                        o_sb = work.tile([128, 64], BF, tag="osb", bufs=4,
                                         name="o_sb")
                        nc.vector.tensor_scalar_mul(
                            o_sb[:, :], pO[h][:, qb, 0:64], den_r[:, :])
                        nc.tensor.transpose(
                            pT[0:64, h * QB + qb, :], o_sb[:, :], id_sb[:, :])

                    def softmax_av(kc, ps_s):
                        off = max(0, kc - noff) * TK
                        j = kc - noff
                        ex = work.tile([128, 2, 512], BF, tag="ex", bufs=6,
                                       name="ex")
                        ps3 = ps_s.rearrange("p (h q) -> p h q", h=2)
                        nc.scalar.activation(
                            ex[:, :, off:], ps3[:, :, off:],
                            mybir.ActivationFunctionType.Exp,
                        )
                        if hp == 0:
                            # just-in-time V chunks, one chunk ahead
                            emit_v(kc)
                            if kc + 1 < kmax:
                                emit_v(kc + 1)
                        for h in (0, 1):
                            if j >= 0:
                                # causal mask on the diagonal 128x128 block
                                nc.vector.tensor_mul(
                                    ex[:, h, off:off + TK],
                                    ex[:, h, off:off + TK],
                                    mask_sb[:, :],
                                )
                            for qb in range(QB):
                                if j > qb:
                                    continue
                                # start=True clears has_written for the WHOLE
                                # psum bank, so only the first matmul per bank
                                # may set it; later first-writes of other
                                # regions overwrite via has_written=0.
                                nc.tensor.matmul(
                                    psO[h][:, qb, :],
                                    ex[:, h, qb * TK:(qb + 1) * TK],
                                    vext[:, kc, hp * 2 + h, :],
                                    start=(kc == 0 and qb == 0),
                                    stop=(kc == noff + qb),
                                    skip_group_check=True,
                                )
                            if j >= 0:
                                # head-0 evict overlaps head-1's AV matmuls
                                evict(h, j, psO, psT)

                    prev = qk(0)
                    for kc in range(kmax):
                        nxt = qk(kc + 1) if kc + 1 < kmax else None
                        if kc + 1 < kmax:
                            # pieces land between scores and the
                            # ACT-dependent AV, hiding the exp latency
                            pump(1)
                        softmax_av(kc, prev)
                        prev = nxt

                    for h in (0, 1):
                        nc.vector.tensor_copy(
                            outT[h * 64:(h + 1) * 64, hp,
                                 qg * TQ:(qg + 1) * TQ],
                            psT[0:64, h * QB:(h + 1) * QB, :],
                        )

            # ---- phase 3: remaining y blocks (sc pool is free now, so
            # double-buffer the leftovers instead of serializing on pq) ----
            pump(len(queue))
            for tk in range(NKC):
                todo = [nb for nb in range(2) if (tk, nb) not in y_done]
                if not todo:
                    continue
                ps_y = pp.tile([128, 1024], F32, tag="sc", bufs=2,
                               name="ps_yL")
                for nb in todo:
                    y_done.add((tk, nb))
                    for cc in range(4):
                        nc.tensor.matmul(
                            ps_y[:, nb * 512:(nb + 1) * 512],
                            outT[:, cc, tk * 128:(tk + 1) * 128],
                            wp_sb[:, cc, nb * 512:(nb + 1) * 512],
                            start=(cc == 0),
                            stop=(cc == 3),
                            skip_group_check=True,
                        )
                y_ev = work.tile([128, 1024], BF, tag="yev", bufs=3,
                                 name="y_ev")
                for nb in todo:
                    sl = slice(nb * 512, (nb + 1) * 512)
                    if (tk + nb) % 2 == 0:
                        nc.vector.tensor_copy(y_ev[:, sl], ps_y[:, sl])
                        nc.sync.dma_start(
                            y[tk * 128:(tk + 1) * 128, sl], y_ev[:, sl])
                    else:
                        nc.scalar.copy(y_ev[:, sl], ps_y[:, sl])
                        nc.gpsimd.dma_start(
                            y[tk * 128:(tk + 1) * 128, sl], y_ev[:, sl])

    nc.compile()
    return nc


_NC_CACHE = None


def _get_nc():
    global _NC_CACHE
    if _NC_CACHE is None:
        _NC_CACHE = build_nc()
    return _NC_CACHE


def make_in_maps(x, w_qkv, w_proj):
    """Host-side sharding: core c -> (batch c//2, head-group c%2)."""
    scale = np.float32(1.0 / np.sqrt(DH))
    ident = np.eye(128, dtype=BF_NP)
    in_maps = []
    for c in range(N_CORES):
        b, g = divmod(c, 2)
        sl = slice(g * 512, (g + 1) * 512)
        xT = np.ascontiguousarray(x[b].T).astype(BF_NP)
        wq = (w_qkv[:, 0 * D:1 * D][:, sl] * scale).astype(BF_NP)
        wk = w_qkv[:, 1 * D:2 * D][:, sl].astype(BF_NP)
        wv = w_qkv[:, 2 * D:3 * D][:, sl].astype(BF_NP)
        wp = np.ascontiguousarray(w_proj[sl, :]).astype(BF_NP)
        in_maps.append({"xT": xT, "wq": wq, "wk": wk, "wv": wv, "wp": wp,
                        "ident": ident})
    return in_maps


def kernel(x, w_qkv, w_proj, _trace=False, _tmpdir=None):
    x = np.asarray(x, dtype=np.float32)
    w_qkv = np.asarray(w_qkv, dtype=np.float32)
    w_proj = np.asarray(w_proj, dtype=np.float32)
    nc = _get_nc()
    in_maps = make_in_maps(x, w_qkv, w_proj)
    res = run_bass_kernel_spmd(
        nc, in_maps, core_ids=list(range(N_CORES)), trace=_trace, tmpdir=_tmpdir
    )
    out = np.empty((B, T, D), dtype=np.float32)
    for b in range(B):
        out[b] = (res.results[2 * b]["y"].astype(np.float32)
                  + res.results[2 * b + 1]["y"].astype(np.float32))
    if _trace:
        kernel._last_results = res
    return out
